# revision 37
# baseline (speedup 1.0000x reference)
"""GQA (n_group == n_head) causal attention kernel for 8 Trainium2 NeuronCores.

Sharding: core c -> (batch b = c//2, head-half hh = c%2).  Each core computes
Q/K/V projections for its 8 heads over the full sequence, causal attention,
and a partial output projection against its 512 rows of Wo.  The host sums
the two partial outputs per batch (the tensor-parallel reduce), adds bo, and
transposes back.

Precision / engine plan:
  - Q/K/V projections: fp8(e4m3) DoubleRow matmuls in THREE passes
    (x_a@W_a + x_a@W_r + x_r@W_a, where _a = fp8 rounding of the 16x-scaled
    operand and _r = fp8 of its rounding residual at the same scale).  This
    recovers ~bf16-level projection accuracy at half the bf16 matmul cost.
    The 16x weight prescale cancels in softmax (Q*K picks up 16^2 -> folded
    into the exp scale; V's 16 cancels against a denominator ones-column of
    value 16).
  - QK^T: bf16, scoresT tiles [k=128, 2 kt-slots, q<=512] in PSUM.
  - exp: ScalarE, PSUM f32 -> SBUF bf16, scale=0.125/256.
  - causal mask: bf16 multiplies on DVE over the diagonal corners only.
  - PV: bf16 against V~ = [16*V | 16] (row 64 = 16*denominator).
  - normalize: DVE reciprocal + Pool partition_broadcast + DVE multiply.
  - out-proj: bf16 matmuls, DVE copy to SBUF, DMA to DRAM.  bo on the host.

Schedule: the ScalarE exp stream is the critical resource (~154us of work),
so projection tiles for token block j+1 and the out-projection for q block
j-1 are injected one PSUM-tile at a time between attention steps of q block
j, keeping both PE and ScalarE continuously busy.

Nonzero bq/bk/bv are supported by augmenting x with a ones row and the
weights with a bias row (npair=5); the staged problem has zero biases so the
default build uses npair=4.
"""

import os
from collections import deque
from contextlib import ExitStack
from functools import partial

import numpy as np

import concourse.bass as bass
import concourse.mybir as mybir
import concourse.tile as tile
from concourse import bacc
from concourse.bass import ds, ts
from concourse.bass_utils import run_bass_kernel_spmd

B, T, D = 4, 2048, 1024
H, HD = 16, 64
NCORES = 8
HH = H // 2            # heads per core = 8
DH = HH * HD           # head dims per core = 512
TB = 512               # token block (q block, proj block)
WSCALE = 16.0          # host-side weight prescale (fp8 range)
SC_EXP = 0.125 / (WSCALE * WSCALE)   # exp input scale for scores
EXP_BIAS = 0.0         # bf16 exp output needs no range shaping

F32 = mybir.dt.float32
BF16 = mybir.dt.bfloat16
F8 = mybir.dt.float8e4
DR = mybir.MatmulPerfMode.DoubleRow
EXP = mybir.ActivationFunctionType.Exp
HDP = HD + 2           # V~ row dim padded so bf16 head strides stay 4-byte
                       # aligned for ldweights (row 64 = denominator ones)

LAST_RESULTS = None


def _build_nc(npair=4):
    deff = 256 * npair
    nc = bacc.Bacc(
        "TRN2",
        target_bir_lowering=False,
        debug=False,
        enable_asserts=False,
        num_devices=NCORES,
    )

    # all big inputs are pre-arranged on the host to match their SBUF
    # layout exactly: one contiguous DMA per load, no gather patterns
    # index 0 = fp8 rounding of the operand, 1 = fp8 of its residual
    x8d = nc.dram_tensor("x8", [4, 2, 128, 2 * npair * TB], F8, kind="ExternalInput").ap()
    wqd = nc.dram_tensor("wq8", [2, 128, 2 * npair * DH], F8, kind="ExternalInput").ap()
    wkd = nc.dram_tensor("wk8", [2, 128, 2 * npair * DH], F8, kind="ExternalInput").ap()
    wvd = nc.dram_tensor("wv8", [2, 128, 2 * npair * DH], F8, kind="ExternalInput").ap()
    wod = nc.dram_tensor("wo16", [128, 4 * D], BF16, kind="ExternalInput").ap()
    maskd = nc.dram_tensor("masks", [128, 384], BF16, kind="ExternalInput").ap()
    outT = nc.dram_tensor("outT", [D, T], F32, kind="ExternalOutput").ap()

    nd = 2 * npair

    with tile.TileContext(nc) as tc, ExitStack() as ctx:
        res = ctx.enter_context(tc.tile_pool(name="res", bufs=1))
        qt8 = res.tile([128, 4, T], BF16, tag="qt")     # 16*Q^T: chunk hp, head i at partitions 64i+
        kt8 = res.tile([128, 4, T], BF16, tag="kt")
        v8 = res.tile([128, 8, 2, HH, HDP], BF16, tag="v")  # [k, ktp, slot, h, 16*V|16|pad]
        ao = res.tile([128, 4, T], BF16, tag="ao")      # attn outT (normalized)
        wo_sb = res.tile([128, 4, D], BF16, tag="wo")
        wq_sb = res.tile([128, 2, nd, DH], F8, tag="wq")
        wk_sb = res.tile([128, 2, nd, DH], F8, tag="wk")
        wv_sb = res.tile([128, 2, nd, DH], F8, tag="wv")
        mask_sb = res.tile([128, 384], BF16, tag="mask")
        nbias = res.tile([128, 1], F32, tag="nbias")
        warm = res.tile([1, 1], F32, tag="warm")

        m1 = mask_sb[:, 0:128]
        m2 = mask_sb[:, 128:384]

        scp = ctx.enter_context(tc.tile_pool(name="scp", bufs=2, space="PSUM"))
        pvp = ctx.enter_context(tc.tile_pool(name="pvp", bufs=2, space="PSUM"))
        xp = ctx.enter_context(tc.tile_pool(name="xp", bufs=2))
        etp = ctx.enter_context(tc.tile_pool(name="etp", bufs=4))
        rdp = ctx.enter_context(tc.tile_pool(name="rdp", bufs=2))
        bcp = ctx.enter_context(tc.tile_pool(name="bcp", bufs=2))
        ostp = ctx.enter_context(tc.tile_pool(name="ostp", bufs=4))

        def load_x(tb, engine=None):
            xt = xp.tile([128, 2, nd, TB], F8, tag="xt", name=f"xt{tb}")
            eng = engine or nc.gpsimd
            for half in range(2):
                eng.dma_start(
                    out=xt[:, half].rearrange("p c t -> p (c t)"),
                    in_=x8d[tb, half],
                )
            return xt

        nc.gpsimd.memset(nbias, EXP_BIAS)
        # pay the Exp activation-table load during the DMA prologue
        nc.scalar.activation(warm, nbias[0:1, :], EXP, scale=1.0)
        # prologue loads: x(0)/wq on the fast HWDGE queues first (they gate
        # the first projection), everything else behind them
        for half in range(2):
            nc.sync.dma_start(
                out=wq_sb[:, half].rearrange("p c d -> p (c d)"), in_=wqd[half]
            )
        xts = {0: load_x(0, nc.scalar)}
        for half in range(2):
            nc.scalar.dma_start(
                out=wk_sb[:, half].rearrange("p c d -> p (c d)"), in_=wkd[half]
            )
        for half in range(2):
            nc.scalar.dma_start(
                out=wv_sb[:, half].rearrange("p c d -> p (c d)"), in_=wvd[half]
            )
        nc.sync.dma_start(out=mask_sb, in_=maskd)
        # ones column of V~ (value 16, matches the 16*V scaling); the pad
        # rows 65..67 just need to be written with something finite
        nc.gpsimd.memset(v8[:, :, :, :, HD:HDP], WSCALE)

        # PE warm-up: keep the PE busy on throwaway matmuls while the first
        # DMAs land, so real work starts at full clock (p-state ramp)
        junk = res.tile([128, TB], BF16, tag="junk")
        junkw = res.tile([128, 1], BF16, tag="junkw")
        nc.gpsimd.memset(junk, 0.0)
        nc.gpsimd.memset(junkw, 0.0)
        for _ in range(6):
            wps = scp.tile([128, 2, TB], F32, tag="sc", name="wps")
            nc.tensor.matmul(wps[0:1, 0, :], junkw, junk, start=True, stop=True)

        PASSES = ((0, 0), (1, 0), (0, 1))  # (w half, x half): a*a + r*a + a*r

        def proj_qk_tile(w_sb, dst, xt, j, dtp):
            ps = scp.tile([128, 2, TB], F32, tag="sc", name="psqk")
            for half in range(2):
                dt = 2 * dtp + half
                for pi, (wh, xh) in enumerate(PASSES):
                    for p in range(npair):
                        nc.tensor.matmul(
                            ps[:, half, :],
                            w_sb[:, wh, 2 * p : 2 * p + 2, ts(dt, 128)],
                            xt[:, xh, 2 * p : 2 * p + 2, :],
                            start=(pi == 0 and p == 0),
                            stop=(pi == 2 and p == npair - 1),
                            perf_mode=DR,
                        )
            nc.vector.tensor_copy(dst[:, 2 * dtp : 2 * dtp + 2, ts(j, TB)], ps)

        def proj_v_tile(xt, j, ttp):
            ps = scp.tile([128, 2, TB], F32, tag="sc", name="psv")
            for half in range(2):
                tt = 2 * ttp + half
                for pi, (wh, xh) in enumerate(PASSES):
                    for p in range(npair):
                        nc.tensor.matmul(
                            ps[:, half, :],
                            xt[:, xh, 2 * p : 2 * p + 2, ts(tt, 128)],
                            wv_sb[:, wh, 2 * p : 2 * p + 2, :],
                            start=(pi == 0 and p == 0),
                            stop=(pi == 2 and p == npair - 1),
                            perf_mode=DR,
                        )
            nc.vector.tensor_copy(
                v8[:, 2 * j + ttp, :, :, 0:HD],
                ps.rearrange("p s (h d) -> p s h d", d=HD),
            )

        def oproj_tile(qc, dt):
            # half-size (one dt chunk) so a single injection stays ~0.9us
            op = scp.tile([128, 2, TB], F32, tag="sc", name="op")
            for cc in range(4):
                nc.tensor.matmul(
                    op[:, 0, :],
                    wo_sb[:, cc, ts(dt, 128)],
                    ao[:, cc, ts(qc, TB)],
                    start=(cc == 0),
                    stop=(cc == 3),
                )
            ost = ostp.tile([128, TB], F32, tag="ost", name="ost")
            nc.vector.tensor_copy(ost, op[:, 0, :])
            nc.sync.dma_start(out=outT[ts(dt, 128), ts(qc, TB)], in_=ost)

        opart = res.tile([128, 8, TB], F32, tag="opart")

        def oproj3_partial(dt):
            # qc=3 out-proj, head chunks 0..1 only -> SBUF partial
            op = scp.tile([128, 2, TB], F32, tag="sc", name="opp")
            for cc in range(2):
                nc.tensor.matmul(
                    op[:, 0, :],
                    wo_sb[:, cc, ts(dt, 128)],
                    ao[:, cc, ts(3, TB)],
                    start=(cc == 0),
                    stop=(cc == 1),
                )
            nc.vector.tensor_copy(opart[:, dt, :], op[:, 0, :])

        def oproj3_finish(dt):
            # head chunks 2..3 + the saved partial
            op = scp.tile([128, 2, TB], F32, tag="sc", name="opf")
            for cc in range(2, 4):
                nc.tensor.matmul(
                    op[:, 0, :],
                    wo_sb[:, cc, ts(dt, 128)],
                    ao[:, cc, ts(3, TB)],
                    start=(cc == 2),
                    stop=(cc == 3),
                )
            ost = ostp.tile([128, TB], F32, tag="ost", name="ost")
            nc.vector.tensor_add(ost, op[:, 0, :], opart[:, dt, :])
            nc.sync.dma_start(out=outT[ts(dt, 128), ts(3, TB)], in_=ost)

        # prologue: token block 0 projections for the first head-pair chunk;
        # the dtp=1 chunks are deferred into the j=0 injection stream so
        # attention (and the exp stream) starts as early as possible
        # wo/x(1) go on the same queue strictly behind wv so they cannot
        # jump ahead of it in the DMA lane (wo isn't needed until ~40us in)
        nc.scalar.dma_start(out=wo_sb.rearrange("p c d -> p (c d)"), in_=wod)
        xts[1] = load_x(1, nc.scalar)
        proj_qk_tile(wq_sb, qt8, xts[0], 0, 0)
        proj_qk_tile(wk_sb, kt8, xts[0], 0, 0)
        proj_v_tile(xts[0], 0, 0)
        proj_v_tile(xts[0], 0, 1)

        for j in range(4):
            qc = j
            nktp = 2 * (qc + 1)

            inj = deque()
            if j == 0:
                # deferred prologue tiles (hp>=1 of q block 0 needs them; hp=0
                # attention runs meanwhile)
                inj.append(partial(proj_qk_tile, wq_sb, qt8, xts[0], 0, 1))
                inj.append(partial(proj_qk_tile, wk_sb, kt8, xts[0], 0, 1))
            if j < 3:
                for dtp in range(2):
                    inj.append(partial(proj_qk_tile, wq_sb, qt8, xts[j + 1], j + 1, dtp))
                for dtp in range(2):
                    inj.append(partial(proj_qk_tile, wk_sb, kt8, xts[j + 1], j + 1, dtp))
                for ttp in range(2):
                    inj.append(partial(proj_v_tile, xts[j + 1], j + 1, ttp))
            if j > 0:
                for dt in range(8):
                    inj.append(partial(oproj_tile, j - 1, dt))
            if j == 3:
                # first-half partial sums of the last block's out-projection
                # (they only need the hp=0/1 chunks of ao, ready mid-block)
                for dt in range(8):
                    inj.append(partial(oproj3_partial, dt))
            if j < 2:
                xts[j + 2] = load_x(j + 2)

            # last iteration: hold one tile back to bridge the final
            # normalize chain before the epilogue out-projection (keeps PE
            # warm through the tail)
            n_inj = len(inj) - 3 if j == 3 else len(inj)
            total_slots = 4 * nktp
            slot = 0
            injected = 0

            for hp in range(4):
                pv = pvp.tile([HDP, 2, TB], F32, tag="pv", name="pv")

                def emit_pv(ets, q0, ktp):
                    for i in range(2):
                        for s in range(2):
                            sq0 = q0 + 128 * s if 2 * ktp >= 4 * qc else q0
                            nc.tensor.matmul(
                                pv[0 : HD + 1, i, sq0:],
                                v8[:, ktp, s, 2 * hp + i, 0 : HD + 1],
                                ets[i][:, s, sq0:],
                                start=(ktp == 0 and s == 0),
                                stop=(ktp == nktp - 1 and s == 1),
                            )

                pending = None  # PVs lag one step so they never wait on exp
                for ktp in range(nktp):
                    rel = 2 * ktp - 4 * qc
                    diag = rel >= 0
                    q0 = rel * 128 if diag else 0
                    qn = TB - q0
                    ets = []
                    for i in range(2):
                        po = 64 * i
                        sc = scp.tile([128, 2, TB], F32, tag="sc", name="sc")
                        for s in range(2):
                            nc.tensor.matmul(
                                sc[:, s, q0:],
                                kt8[po : po + 64, hp, ds(256 * ktp + 128 * s, 128)],
                                qt8[po : po + 64, hp, ds(qc * TB + q0, qn)],
                                start=True,
                                stop=True,
                            )
                        et = etp.tile([128, 2, TB], BF16, tag="et", name="et")
                        nc.scalar.activation(
                            et[:, :, q0:], sc[:, :, q0:], EXP,
                            scale=SC_EXP, bias=nbias,
                        )
                        if diag:
                            nc.vector.tensor_mul(
                                et[:, 0, ds(q0, 128)], et[:, 0, ds(q0, 128)], m1
                            )
                            nc.vector.tensor_mul(
                                et[:, 1, ds(q0, 256)], et[:, 1, ds(q0, 256)], m2
                            )
                        ets.append(et)
                    # paced injection of independent PE work, while ScalarE
                    # runs the exps (and DVE the diagonal masks)
                    slot += 1
                    target = (n_inj * slot + total_slots - 1) // total_slots
                    while inj and injected < target:
                        inj.popleft()()
                        injected += 1
                    if pending is not None:
                        emit_pv(*pending)
                    pending = (ets, q0, ktp)
                emit_pv(*pending)
                # normalize: rows 0..63 / (row 64), per head
                rd = rdp.tile([1, 2, TB], F32, tag="rd", name="rd")
                nc.vector.reciprocal(rd, pv[HD : HD + 1, :, :])
                bcs = bcp.tile([64, 2, TB], F32, tag="bcs", name="bcs")
                nc.gpsimd.partition_broadcast(bcs, rd)
                for i in range(2):
                    po = 64 * i
                    nc.vector.tensor_mul(
                        ao[po : po + 64, hp, ts(qc, TB)], pv[0:HD, i, :], bcs[:, i, :]
                    )
            while inj:
                inj.popleft()()

        # epilogue: finish the last q block's output projection
        for dt in range(8):
            oproj3_finish(dt)

    nc.compile()
    return nc


def _make_masks():
    # M1: lower-triangular 128x128 (valid iff q >= k); M2: [zeros | M1]
    k = np.arange(128)[:, None]
    q = np.arange(128)[None, :]
    m1 = (q >= k).astype(np.float32)
    m = np.zeros((128, 384), np.float32)
    m[:, 0:128] = m1
    m[:, 256:384] = m1
    return m


def kernel(x, Wq, bq, Wk, bk, Wv, bv, Wo, bo):
    global LAST_RESULTS
    import ml_dtypes

    F8NP = ml_dtypes.float8_e4m3
    BF16NP = ml_dtypes.bfloat16

    x = np.asarray(x, np.float32)
    Wq, bq = np.asarray(Wq, np.float32), np.asarray(bq, np.float32)
    Wk, bk = np.asarray(Wk, np.float32), np.asarray(bk, np.float32)
    Wv, bv = np.asarray(Wv, np.float32), np.asarray(bv, np.float32)
    Wo, bo = np.asarray(Wo, np.float32), np.asarray(bo, np.float32)

    with_bias = bool(np.any(bq) or np.any(bk) or np.any(bv))
    npair = 5 if with_bias else 4
    deff = 256 * npair

    masks = _make_masks().astype(BF16NP)

    def split8(a):
        # fp8 value + fp8 residual at the same scale
        hi = a.astype(F8NP)
        lo = (a - hi.astype(np.float32)).astype(F8NP)
        return hi, lo

    def aug_x(xb):
        # xT (+ ones row at 1024 when biased), pre-tiled to [4, 2, 128, nd*TB]
        xa = np.zeros((deff, T), np.float32)
        xa[:D] = xb.T
        if with_bias:
            xa[D] = 1.0
        # row c*128+p, col tb*TB+t -> [tb, p, c*TB+t]
        xt = xa.reshape(2 * npair, 128, 4, TB).transpose(2, 1, 0, 3)
        xt = np.ascontiguousarray(xt.reshape(4, 128, 2 * npair * TB))
        hi, lo = split8(xt)
        return np.ascontiguousarray(np.stack([hi, lo], axis=1))

    def aug_w(W, b, sl):
        wa = np.zeros((deff, DH), np.float32)
        wa[:D] = W[:, sl] * WSCALE
        if with_bias:
            wa[D] = b[sl] * WSCALE
        wt = wa.reshape(2 * npair, 128, DH).transpose(1, 0, 2)
        wt = np.ascontiguousarray(wt.reshape(128, 2 * npair * DH))
        hi, lo = split8(wt)
        return np.ascontiguousarray(np.stack([hi, lo], axis=0))

    in_maps = []
    for c in range(NCORES):
        b, hh = c // 2, c % 2
        sl = slice(hh * DH, (hh + 1) * DH)
        in_maps.append(
            {
                "x8": aug_x(x[b]),
                "wq8": aug_w(Wq, bq, sl),
                "wk8": aug_w(Wk, bk, sl),
                "wv8": aug_w(Wv, bv, sl),
                "wo16": np.ascontiguousarray(
                    Wo[sl, :].reshape(4, 128, D).transpose(1, 0, 2).reshape(128, 4 * D)
                ).astype(BF16NP),
                "masks": masks,
            }
        )

    nc = _build_nc(npair)
    res = run_bass_kernel_spmd(
        nc,
        in_maps,
        core_ids=list(range(NCORES)),
        trace=bool(int(os.environ.get("KERNEL_TRACE", "0"))),
    )
    LAST_RESULTS = res

    out = np.empty((B, T, D), np.float32)
    for b in range(B):
        acc = res.results[2 * b]["outT"] + res.results[2 * b + 1]["outT"]
        out[b] = acc.T + bo
    return out


# revision 43
# speedup vs baseline: 1.0029x; 1.0029x over previous
"""GQA (n_group == n_head) causal attention kernel for 8 Trainium2 NeuronCores.

Sharding: core c -> (batch b = c//2, head-half hh = c%2).  Each core computes
Q/K/V projections for its 8 heads over the full sequence, causal attention,
and a partial output projection against its 512 rows of Wo.  The host sums
the two partial outputs per batch (the tensor-parallel reduce), adds bo, and
transposes back.

Precision / engine plan:
  - Q/K/V projections: fp8(e4m3) DoubleRow matmuls in THREE passes
    (x_a@W_a + x_a@W_r + x_r@W_a, where _a = fp8 rounding of the 16x-scaled
    operand and _r = fp8 of its rounding residual at the same scale).  This
    recovers ~bf16-level projection accuracy at half the bf16 matmul cost.
    The 16x weight prescale cancels in softmax (Q*K picks up 16^2 -> folded
    into the exp scale; V's 16 cancels against a denominator ones-column of
    value 16).
  - QK^T: bf16, scoresT tiles [k=128, 2 kt-slots, q<=512] in PSUM.
  - exp: ScalarE, PSUM f32 -> SBUF bf16, scale=0.125/256.
  - causal mask: bf16 multiplies on DVE over the diagonal corners only.
  - PV: bf16 against V~ = [16*V | 16] (row 64 = 16*denominator).
  - normalize: DVE reciprocal + Pool partition_broadcast + DVE multiply.
  - out-proj: bf16 matmuls, DVE copy to SBUF, DMA to DRAM.  bo on the host.

Schedule: the ScalarE exp stream is the critical resource (~154us of work),
so projection tiles for token block j+1 and the out-projection for q block
j-1 are injected one PSUM-tile at a time between attention steps of q block
j, keeping both PE and ScalarE continuously busy.

Nonzero bq/bk/bv are supported by augmenting x with a ones row and the
weights with a bias row (npair=5); the staged problem has zero biases so the
default build uses npair=4.
"""

import os
from collections import deque
from contextlib import ExitStack
from functools import partial

import numpy as np

import concourse.bass as bass
import concourse.mybir as mybir
import concourse.tile as tile
from concourse import bacc
from concourse.bass import ds, ts
from concourse.bass_utils import run_bass_kernel_spmd

B, T, D = 4, 2048, 1024
H, HD = 16, 64
NCORES = 8
HH = H // 2            # heads per core = 8
DH = HH * HD           # head dims per core = 512
TB = 512               # token block (q block, proj block)
WSCALE = 16.0          # host-side weight prescale (fp8 range)
SC_EXP = 0.125 / (WSCALE * WSCALE)   # exp input scale for scores
EXP_BIAS = 0.0         # bf16 exp output needs no range shaping

F32 = mybir.dt.float32
BF16 = mybir.dt.bfloat16
F8 = mybir.dt.float8e4
DR = mybir.MatmulPerfMode.DoubleRow
EXP = mybir.ActivationFunctionType.Exp
HDP = HD + 2           # V~ row dim padded so bf16 head strides stay 4-byte
                       # aligned for ldweights (row 64 = denominator ones)

LAST_RESULTS = None


def _build_nc(npair=4):
    deff = 256 * npair
    nc = bacc.Bacc(
        "TRN2",
        target_bir_lowering=False,
        debug=False,
        enable_asserts=False,
        num_devices=NCORES,
    )

    # all big inputs are pre-arranged on the host to match their SBUF
    # layout exactly: one contiguous DMA per load, no gather patterns
    # index 0 = fp8 rounding of the operand, 1 = fp8 of its residual
    x8d = nc.dram_tensor("x8", [4, 2, 128, 2 * npair * TB], F8, kind="ExternalInput").ap()
    wqd = nc.dram_tensor("wq8", [2, 128, 2 * npair * DH], F8, kind="ExternalInput").ap()
    wkd = nc.dram_tensor("wk8", [2, 128, 2 * npair * DH], F8, kind="ExternalInput").ap()
    wvd = nc.dram_tensor("wv8", [2, 128, 2 * npair * DH], F8, kind="ExternalInput").ap()
    wod = nc.dram_tensor("wo16", [128, 4 * D], BF16, kind="ExternalInput").ap()
    maskd = nc.dram_tensor("masks", [128, 384], BF16, kind="ExternalInput").ap()
    outT = nc.dram_tensor("outT", [D, T], BF16, kind="ExternalOutput").ap()

    nd = 2 * npair

    with tile.TileContext(nc) as tc, ExitStack() as ctx:
        res = ctx.enter_context(tc.tile_pool(name="res", bufs=1))
        qt8 = res.tile([128, 4, T], BF16, tag="qt")     # 16*Q^T: chunk hp, head i at partitions 64i+
        kt8 = res.tile([128, 4, T], BF16, tag="kt")
        v8 = res.tile([128, 8, 2, HH, HDP], BF16, tag="v")  # [k, ktp, slot, h, 16*V|16|pad]
        ao = res.tile([128, 4, T], BF16, tag="ao")      # attn outT (normalized)
        wo_sb = res.tile([128, 4, D], BF16, tag="wo")
        wq_sb = res.tile([128, 2, nd, DH], F8, tag="wq")
        wk_sb = res.tile([128, 2, nd, DH], F8, tag="wk")
        wv_sb = res.tile([128, 2, nd, DH], F8, tag="wv")
        mask_sb = res.tile([128, 384], BF16, tag="mask")
        nbias = res.tile([128, 1], F32, tag="nbias")
        warm = res.tile([1, 1], F32, tag="warm")

        m1 = mask_sb[:, 0:128]
        m2 = mask_sb[:, 128:384]

        scp = ctx.enter_context(tc.tile_pool(name="scp", bufs=2, space="PSUM"))
        pvp = ctx.enter_context(tc.tile_pool(name="pvp", bufs=2, space="PSUM"))
        xp = ctx.enter_context(tc.tile_pool(name="xp", bufs=2))
        etp = ctx.enter_context(tc.tile_pool(name="etp", bufs=6))
        rdp = ctx.enter_context(tc.tile_pool(name="rdp", bufs=2))
        bcp = ctx.enter_context(tc.tile_pool(name="bcp", bufs=2))
        ostp = ctx.enter_context(tc.tile_pool(name="ostp", bufs=4))

        def load_x(tb, engine=None):
            xt = xp.tile([128, 2, nd, TB], F8, tag="xt", name=f"xt{tb}")
            eng = engine or nc.gpsimd
            for half in range(2):
                eng.dma_start(
                    out=xt[:, half].rearrange("p c t -> p (c t)"),
                    in_=x8d[tb, half],
                )
            return xt

        nc.gpsimd.memset(nbias, EXP_BIAS)
        # pay the Exp activation-table load during the DMA prologue
        nc.scalar.activation(warm, nbias[0:1, :], EXP, scale=1.0)
        # prologue loads: x(0)/wq on the fast HWDGE queues first (they gate
        # the first projection), everything else behind them
        for half in range(2):
            nc.sync.dma_start(
                out=wq_sb[:, half].rearrange("p c d -> p (c d)"), in_=wqd[half]
            )
        xts = {0: load_x(0, nc.scalar)}
        for half in range(2):
            nc.scalar.dma_start(
                out=wk_sb[:, half].rearrange("p c d -> p (c d)"), in_=wkd[half]
            )
        for half in range(2):
            nc.scalar.dma_start(
                out=wv_sb[:, half].rearrange("p c d -> p (c d)"), in_=wvd[half]
            )
        nc.sync.dma_start(out=mask_sb, in_=maskd)
        # ones column of V~ (value 16, matches the 16*V scaling); the pad
        # rows 65..67 just need to be written with something finite
        nc.gpsimd.memset(v8[:, :, :, :, HD:HDP], WSCALE)

        # PE warm-up: keep the PE busy on throwaway matmuls while the first
        # DMAs land, so real work starts at full clock (p-state ramp)
        junk = res.tile([128, TB], BF16, tag="junk")
        junkw = res.tile([128, 1], BF16, tag="junkw")
        nc.gpsimd.memset(junk, 0.0)
        nc.gpsimd.memset(junkw, 0.0)
        for _ in range(6):
            wps = scp.tile([128, 2, TB], F32, tag="sc", name="wps")
            nc.tensor.matmul(wps[0:1, 0, :], junkw, junk, start=True, stop=True)

        PASSES = ((0, 0), (1, 0), (0, 1))  # (w half, x half): a*a + r*a + a*r

        def proj_qk_tile(w_sb, dst, xt, j, dtp):
            ps = scp.tile([128, 2, TB], F32, tag="sc", name="psqk")
            for half in range(2):
                dt = 2 * dtp + half
                for pi, (wh, xh) in enumerate(PASSES):
                    for p in range(npair):
                        nc.tensor.matmul(
                            ps[:, half, :],
                            w_sb[:, wh, 2 * p : 2 * p + 2, ts(dt, 128)],
                            xt[:, xh, 2 * p : 2 * p + 2, :],
                            start=(pi == 0 and p == 0),
                            stop=(pi == 2 and p == npair - 1),
                            perf_mode=DR,
                        )
            nc.vector.tensor_copy(dst[:, 2 * dtp : 2 * dtp + 2, ts(j, TB)], ps)

        def proj_v_tile(xt, j, ttp):
            ps = scp.tile([128, 2, TB], F32, tag="sc", name="psv")
            for half in range(2):
                tt = 2 * ttp + half
                for pi, (wh, xh) in enumerate(PASSES):
                    for p in range(npair):
                        nc.tensor.matmul(
                            ps[:, half, :],
                            xt[:, xh, 2 * p : 2 * p + 2, ts(tt, 128)],
                            wv_sb[:, wh, 2 * p : 2 * p + 2, :],
                            start=(pi == 0 and p == 0),
                            stop=(pi == 2 and p == npair - 1),
                            perf_mode=DR,
                        )
            nc.vector.tensor_copy(
                v8[:, 2 * j + ttp, :, :, 0:HD],
                ps.rearrange("p s (h d) -> p s h d", d=HD),
            )

        def oproj_tile(qc, dt):
            # half-size (one dt chunk) so a single injection stays ~0.9us
            op = scp.tile([128, 2, TB], F32, tag="sc", name="op")
            for cc in range(4):
                nc.tensor.matmul(
                    op[:, 0, :],
                    wo_sb[:, cc, ts(dt, 128)],
                    ao[:, cc, ts(qc, TB)],
                    start=(cc == 0),
                    stop=(cc == 3),
                )
            ost = ostp.tile([128, TB], BF16, tag="ost", name="ost")
            nc.vector.tensor_copy(ost, op[:, 0, :])
            nc.sync.dma_start(out=outT[ts(dt, 128), ts(qc, TB)], in_=ost)

        opart = res.tile([128, 8, TB], F32, tag="opart")

        def oproj3_partial(dt):
            # qc=3 out-proj, head chunks 0..1 only -> SBUF partial
            op = scp.tile([128, 2, TB], F32, tag="sc", name="opp")
            for cc in range(2):
                nc.tensor.matmul(
                    op[:, 0, :],
                    wo_sb[:, cc, ts(dt, 128)],
                    ao[:, cc, ts(3, TB)],
                    start=(cc == 0),
                    stop=(cc == 1),
                )
            nc.vector.tensor_copy(opart[:, dt, :], op[:, 0, :])

        def oproj3_finish(dt):
            # head chunks 2..3 + the saved partial
            op = scp.tile([128, 2, TB], F32, tag="sc", name="opf")
            for cc in range(2, 4):
                nc.tensor.matmul(
                    op[:, 0, :],
                    wo_sb[:, cc, ts(dt, 128)],
                    ao[:, cc, ts(3, TB)],
                    start=(cc == 2),
                    stop=(cc == 3),
                )
            ost = ostp.tile([128, TB], BF16, tag="ost", name="ost")
            nc.vector.tensor_add(ost, op[:, 0, :], opart[:, dt, :])
            nc.sync.dma_start(out=outT[ts(dt, 128), ts(3, TB)], in_=ost)

        # prologue: token block 0 projections for the first head-pair chunk;
        # the dtp=1 chunks are deferred into the j=0 injection stream so
        # attention (and the exp stream) starts as early as possible
        # wo/x(1) go on the same queue strictly behind wv so they cannot
        # jump ahead of it in the DMA lane (wo isn't needed until ~40us in)
        nc.scalar.dma_start(out=wo_sb.rearrange("p c d -> p (c d)"), in_=wod)
        xts[1] = load_x(1, nc.scalar)
        proj_qk_tile(wq_sb, qt8, xts[0], 0, 0)
        proj_qk_tile(wk_sb, kt8, xts[0], 0, 0)
        proj_v_tile(xts[0], 0, 0)
        proj_v_tile(xts[0], 0, 1)

        for j in range(4):
            qc = j
            nktp = 2 * (qc + 1)

            inj = deque()
            if j == 0:
                # deferred prologue tiles (hp>=1 of q block 0 needs them; hp=0
                # attention runs meanwhile)
                inj.append(partial(proj_qk_tile, wq_sb, qt8, xts[0], 0, 1))
                inj.append(partial(proj_qk_tile, wk_sb, kt8, xts[0], 0, 1))
            if j < 3:
                for dtp in range(2):
                    inj.append(partial(proj_qk_tile, wq_sb, qt8, xts[j + 1], j + 1, dtp))
                for dtp in range(2):
                    inj.append(partial(proj_qk_tile, wk_sb, kt8, xts[j + 1], j + 1, dtp))
                for ttp in range(2):
                    inj.append(partial(proj_v_tile, xts[j + 1], j + 1, ttp))
            if j > 0:
                for dt in range(8):
                    inj.append(partial(oproj_tile, j - 1, dt))
            if j == 3:
                # first-half partial sums of the last block's out-projection
                # (they only need the hp=0/1 chunks of ao, ready mid-block)
                for dt in range(8):
                    inj.append(partial(oproj3_partial, dt))
            if j < 2:
                xts[j + 2] = load_x(j + 2)

            # last iteration: hold one tile back to bridge the final
            # normalize chain before the epilogue out-projection (keeps PE
            # warm through the tail)
            n_inj = len(inj) - 3 if j == 3 else len(inj)
            total_slots = 4 * nktp
            slot = 0
            injected = 0

            for hp in range(4):
                pv = pvp.tile([HDP, 2, TB], F32, tag="pv", name="pv")

                def emit_pv(ets, q0, ktp):
                    for i in range(2):
                        for s in range(2):
                            sq0 = q0 + 128 * s if 2 * ktp >= 4 * qc else q0
                            nc.tensor.matmul(
                                pv[0 : HD + 1, i, sq0:],
                                v8[:, ktp, s, 2 * hp + i, 0 : HD + 1],
                                ets[i][:, s, sq0:],
                                start=(ktp == 0 and s == 0),
                                stop=(ktp == nktp - 1 and s == 1),
                            )

                pending = None  # PVs lag one step so they never wait on exp
                for ktp in range(nktp):
                    rel = 2 * ktp - 4 * qc
                    diag = rel >= 0
                    q0 = rel * 128 if diag else 0
                    qn = TB - q0
                    ets = []
                    for i in range(2):
                        po = 64 * i
                        sc = scp.tile([128, 2, TB], F32, tag="sc", name="sc")
                        for s in range(2):
                            nc.tensor.matmul(
                                sc[:, s, q0:],
                                kt8[po : po + 64, hp, ds(256 * ktp + 128 * s, 128)],
                                qt8[po : po + 64, hp, ds(qc * TB + q0, qn)],
                                start=True,
                                stop=True,
                            )
                        et = etp.tile([128, 2, TB], BF16, tag="et", name="et")
                        nc.scalar.activation(
                            et[:, :, q0:], sc[:, :, q0:], EXP,
                            scale=SC_EXP, bias=nbias,
                        )
                        if diag:
                            nc.vector.tensor_mul(
                                et[:, 0, ds(q0, 128)], et[:, 0, ds(q0, 128)], m1
                            )
                            nc.vector.tensor_mul(
                                et[:, 1, ds(q0, 256)], et[:, 1, ds(q0, 256)], m2
                            )
                        ets.append(et)
                    # paced injection of independent PE work, while ScalarE
                    # runs the exps (and DVE the diagonal masks)
                    slot += 1
                    target = (n_inj * slot + total_slots - 1) // total_slots
                    while inj and injected < target:
                        inj.popleft()()
                        injected += 1
                    if pending is not None:
                        emit_pv(*pending)
                    pending = (ets, q0, ktp)
                emit_pv(*pending)
                # normalize: rows 0..63 / (row 64), per head
                rd = rdp.tile([1, 2, TB], F32, tag="rd", name="rd")
                nc.vector.reciprocal(rd, pv[HD : HD + 1, :, :])
                bcs = bcp.tile([64, 2, TB], F32, tag="bcs", name="bcs")
                nc.gpsimd.partition_broadcast(bcs, rd)
                for i in range(2):
                    po = 64 * i
                    nc.vector.tensor_mul(
                        ao[po : po + 64, hp, ts(qc, TB)], pv[0:HD, i, :], bcs[:, i, :]
                    )
            while inj:
                inj.popleft()()

        # epilogue: finish the last q block's output projection
        for dt in range(8):
            oproj3_finish(dt)

    nc.compile()
    return nc


def _make_masks():
    # M1: lower-triangular 128x128 (valid iff q >= k); M2: [zeros | M1]
    k = np.arange(128)[:, None]
    q = np.arange(128)[None, :]
    m1 = (q >= k).astype(np.float32)
    m = np.zeros((128, 384), np.float32)
    m[:, 0:128] = m1
    m[:, 256:384] = m1
    return m


def kernel(x, Wq, bq, Wk, bk, Wv, bv, Wo, bo):
    global LAST_RESULTS
    import ml_dtypes

    F8NP = ml_dtypes.float8_e4m3
    BF16NP = ml_dtypes.bfloat16

    x = np.asarray(x, np.float32)
    Wq, bq = np.asarray(Wq, np.float32), np.asarray(bq, np.float32)
    Wk, bk = np.asarray(Wk, np.float32), np.asarray(bk, np.float32)
    Wv, bv = np.asarray(Wv, np.float32), np.asarray(bv, np.float32)
    Wo, bo = np.asarray(Wo, np.float32), np.asarray(bo, np.float32)

    with_bias = bool(np.any(bq) or np.any(bk) or np.any(bv))
    npair = 5 if with_bias else 4
    deff = 256 * npair

    masks = _make_masks().astype(BF16NP)

    def split8(a):
        # fp8 value + fp8 residual at the same scale
        hi = a.astype(F8NP)
        lo = (a - hi.astype(np.float32)).astype(F8NP)
        return hi, lo

    def aug_x(xb):
        # xT (+ ones row at 1024 when biased), pre-tiled to [4, 2, 128, nd*TB]
        xa = np.zeros((deff, T), np.float32)
        xa[:D] = xb.T
        if with_bias:
            xa[D] = 1.0
        # row c*128+p, col tb*TB+t -> [tb, p, c*TB+t]
        xt = xa.reshape(2 * npair, 128, 4, TB).transpose(2, 1, 0, 3)
        xt = np.ascontiguousarray(xt.reshape(4, 128, 2 * npair * TB))
        hi, lo = split8(xt)
        return np.ascontiguousarray(np.stack([hi, lo], axis=1))

    def aug_w(W, b, sl):
        wa = np.zeros((deff, DH), np.float32)
        wa[:D] = W[:, sl] * WSCALE
        if with_bias:
            wa[D] = b[sl] * WSCALE
        wt = wa.reshape(2 * npair, 128, DH).transpose(1, 0, 2)
        wt = np.ascontiguousarray(wt.reshape(128, 2 * npair * DH))
        hi, lo = split8(wt)
        return np.ascontiguousarray(np.stack([hi, lo], axis=0))

    in_maps = []
    for c in range(NCORES):
        b, hh = c // 2, c % 2
        sl = slice(hh * DH, (hh + 1) * DH)
        in_maps.append(
            {
                "x8": aug_x(x[b]),
                "wq8": aug_w(Wq, bq, sl),
                "wk8": aug_w(Wk, bk, sl),
                "wv8": aug_w(Wv, bv, sl),
                "wo16": np.ascontiguousarray(
                    Wo[sl, :].reshape(4, 128, D).transpose(1, 0, 2).reshape(128, 4 * D)
                ).astype(BF16NP),
                "masks": masks,
            }
        )

    nc = _build_nc(npair)
    res = run_bass_kernel_spmd(
        nc,
        in_maps,
        core_ids=list(range(NCORES)),
        trace=bool(int(os.environ.get("KERNEL_TRACE", "0"))),
    )
    LAST_RESULTS = res

    out = np.empty((B, T, D), np.float32)
    for b in range(B):
        acc = (
            res.results[2 * b]["outT"].astype(np.float32)
            + res.results[2 * b + 1]["outT"].astype(np.float32)
        )
        out[b] = acc.T + bo
    return out


# revision 51
# speedup vs baseline: 1.0113x; 1.0084x over previous
"""GQA (n_group == n_head) causal attention kernel for 8 Trainium2 NeuronCores.

Sharding: core c -> (batch b = c//2, head-half hh = c%2).  Each core computes
Q/K/V projections for its 8 heads over the full sequence, causal attention,
and a partial output projection against its 512 rows of Wo.  The host sums
the two partial outputs per batch (the tensor-parallel reduce), adds bo, and
transposes back.

Precision / engine plan:
  - Q/K/V projections: fp8(e4m3) DoubleRow matmuls in THREE passes
    (x_a@W_a + x_a@W_r + x_r@W_a, where _a = fp8 rounding of the 16x-scaled
    operand and _r = fp8 of its rounding residual at the same scale).  This
    recovers ~bf16-level projection accuracy at half the bf16 matmul cost.
    The 16x weight prescale cancels in softmax (Q*K picks up 16^2 -> folded
    into the exp scale; V's 16 cancels against a denominator ones-column of
    value 16).
  - QK^T: bf16, scoresT tiles [k=128, 2 kt-slots, q<=512] in PSUM.
  - exp: ScalarE, PSUM f32 -> SBUF bf16, scale=0.125/256.
  - causal mask: bf16 multiplies on DVE over the diagonal corners only.
  - PV: bf16 against V~ = [16*V | 16] (row 64 = 16*denominator).
  - normalize: DVE reciprocal + Pool partition_broadcast + DVE multiply.
  - out-proj: bf16 matmuls, DVE copy to SBUF, DMA to DRAM.  bo on the host.

Schedule: the ScalarE exp stream is the critical resource (~154us of work),
so projection tiles for token block j+1 and the out-projection for q block
j-1 are injected one PSUM-tile at a time between attention steps of q block
j, keeping both PE and ScalarE continuously busy.

Nonzero bq/bk/bv are supported by augmenting x with a ones row and the
weights with a bias row (npair=5); the staged problem has zero biases so the
default build uses npair=4.
"""

import os
from collections import deque
from contextlib import ExitStack
from functools import partial

import numpy as np

import concourse.bass as bass
import concourse.mybir as mybir
import concourse.tile as tile
from concourse import bacc
from concourse.bass import ds, ts
from concourse.bass_utils import run_bass_kernel_spmd

B, T, D = 4, 2048, 1024
H, HD = 16, 64
NCORES = 8
HH = H // 2            # heads per core = 8
DH = HH * HD           # head dims per core = 512
TB = 512               # token block (q block, proj block)
WSCALE = 16.0          # host-side weight prescale (fp8 range)
SC_EXP = 0.125 / (WSCALE * WSCALE)   # exp input scale for scores
EXP_BIAS = 0.0         # bf16 exp output needs no range shaping

F32 = mybir.dt.float32
BF16 = mybir.dt.bfloat16
F8 = mybir.dt.float8e4
DR = mybir.MatmulPerfMode.DoubleRow
EXP = mybir.ActivationFunctionType.Exp
HDP = HD + 2           # V~ row dim padded so bf16 head strides stay 4-byte
                       # aligned for ldweights (row 64 = denominator ones)

LAST_RESULTS = None


def _build_nc(npair=4):
    deff = 256 * npair
    nc = bacc.Bacc(
        "TRN2",
        target_bir_lowering=False,
        debug=False,
        enable_asserts=False,
        num_devices=NCORES,
    )

    # all big inputs are pre-arranged on the host to match their SBUF
    # layout exactly: one contiguous DMA per load, no gather patterns
    # index 0 = fp8 rounding of the operand, 1 = fp8 of its residual
    x8d = nc.dram_tensor("x8", [4, 2, 128, 2 * npair * TB], F8, kind="ExternalInput").ap()
    wqd = nc.dram_tensor("wq8", [2, 128, 2 * npair * DH], F8, kind="ExternalInput").ap()
    wkd = nc.dram_tensor("wk8", [2, 128, 2 * npair * DH], F8, kind="ExternalInput").ap()
    wvd = nc.dram_tensor("wv8", [2, 128, 2 * npair * DH], F8, kind="ExternalInput").ap()
    wod = nc.dram_tensor("wo16", [128, 4 * D], BF16, kind="ExternalInput").ap()
    maskd = nc.dram_tensor("masks", [128, 384], BF16, kind="ExternalInput").ap()
    outT = nc.dram_tensor("outT", [D, T], BF16, kind="ExternalOutput").ap()

    nd = 2 * npair

    with tile.TileContext(nc) as tc, ExitStack() as ctx:
        res = ctx.enter_context(tc.tile_pool(name="res", bufs=1))
        qt8 = res.tile([128, 4, T], BF16, tag="qt")     # 16*Q^T: chunk hp, head i at partitions 64i+
        kt8 = res.tile([128, 4, T], BF16, tag="kt")
        v8 = res.tile([128, 8, 2, HH, HDP], BF16, tag="v")  # [k, ktp, slot, h, 16*V|16|pad]
        ao = res.tile([128, 4, T], BF16, tag="ao")      # attn outT (normalized)
        wo_sb = res.tile([128, 4, D], BF16, tag="wo")
        wq_sb = res.tile([128, 2, nd, DH], F8, tag="wq")
        wk_sb = res.tile([128, 2, nd, DH], F8, tag="wk")
        wv_sb = res.tile([128, 2, nd, DH], F8, tag="wv")
        mask_sb = res.tile([128, 384], BF16, tag="mask")
        nbias = res.tile([128, 1], F32, tag="nbias")
        warm = res.tile([1, 1], F32, tag="warm")

        m1 = mask_sb[:, 0:128]
        m2 = mask_sb[:, 128:384]

        scp = ctx.enter_context(tc.tile_pool(name="scp", bufs=2, space="PSUM"))
        pvp = ctx.enter_context(tc.tile_pool(name="pvp", bufs=2, space="PSUM"))
        xp = ctx.enter_context(tc.tile_pool(name="xp", bufs=2))
        etp = ctx.enter_context(tc.tile_pool(name="etp", bufs=6))
        rdp = ctx.enter_context(tc.tile_pool(name="rdp", bufs=2))
        bcp = ctx.enter_context(tc.tile_pool(name="bcp", bufs=2))
        ostp = ctx.enter_context(tc.tile_pool(name="ostp", bufs=6))

        def load_x(tb, engine=None):
            xt = xp.tile([128, 2, nd, TB], F8, tag="xt", name=f"xt{tb}")
            eng = engine or nc.gpsimd
            for half in range(2):
                eng.dma_start(
                    out=xt[:, half].rearrange("p c t -> p (c t)"),
                    in_=x8d[tb, half],
                )
            return xt

        nc.gpsimd.memset(nbias, EXP_BIAS)
        # pay the Exp activation-table load during the DMA prologue
        nc.scalar.activation(warm, nbias[0:1, :], EXP, scale=1.0)
        # prologue loads: x(0)/wq on the fast HWDGE queues first (they gate
        # the first projection), everything else behind them
        for half in range(2):
            nc.sync.dma_start(
                out=wq_sb[:, half].rearrange("p c d -> p (c d)"), in_=wqd[half]
            )
        xts = {0: load_x(0, nc.scalar)}
        for half in range(2):
            nc.scalar.dma_start(
                out=wk_sb[:, half].rearrange("p c d -> p (c d)"), in_=wkd[half]
            )
        for half in range(2):
            nc.scalar.dma_start(
                out=wv_sb[:, half].rearrange("p c d -> p (c d)"), in_=wvd[half]
            )
        nc.sync.dma_start(out=mask_sb, in_=maskd)
        # ones column of V~ (value 16, matches the 16*V scaling); the pad
        # rows 65..67 just need to be written with something finite
        nc.gpsimd.memset(v8[:, :, :, :, HD:HDP], WSCALE)

        # PE warm-up: keep the PE busy on throwaway matmuls while the first
        # DMAs land, so real work starts at full clock (p-state ramp)
        junk = res.tile([128, TB], BF16, tag="junk")
        junkw = res.tile([128, 1], BF16, tag="junkw")
        nc.gpsimd.memset(junk, 0.0)
        nc.gpsimd.memset(junkw, 0.0)
        for _ in range(6):
            wps = scp.tile([128, 2, TB], F32, tag="sc", name="wps")
            nc.tensor.matmul(wps[0:1, 0, :], junkw, junk, start=True, stop=True)

        PASSES = ((0, 0), (1, 0), (0, 1))  # (w half, x half): a*a + r*a + a*r

        def proj_qk_tile(w_sb, dst, xt, j, dtp):
            ps = scp.tile([128, 2, TB], F32, tag="sc", name="psqk")
            for half in range(2):
                dt = 2 * dtp + half
                for pi, (wh, xh) in enumerate(PASSES):
                    for p in range(npair):
                        nc.tensor.matmul(
                            ps[:, half, :],
                            w_sb[:, wh, 2 * p : 2 * p + 2, ts(dt, 128)],
                            xt[:, xh, 2 * p : 2 * p + 2, :],
                            start=(pi == 0 and p == 0),
                            stop=(pi == 2 and p == npair - 1),
                            perf_mode=DR,
                        )
            nc.vector.tensor_copy(dst[:, 2 * dtp : 2 * dtp + 2, ts(j, TB)], ps)

        def proj_v_tile(xt, j, ttp):
            ps = scp.tile([128, 2, TB], F32, tag="sc", name="psv")
            for half in range(2):
                tt = 2 * ttp + half
                for pi, (wh, xh) in enumerate(PASSES):
                    for p in range(npair):
                        nc.tensor.matmul(
                            ps[:, half, :],
                            xt[:, xh, 2 * p : 2 * p + 2, ts(tt, 128)],
                            wv_sb[:, wh, 2 * p : 2 * p + 2, :],
                            start=(pi == 0 and p == 0),
                            stop=(pi == 2 and p == npair - 1),
                            perf_mode=DR,
                        )
            nc.vector.tensor_copy(
                v8[:, 2 * j + ttp, :, :, 0:HD],
                ps.rearrange("p s (h d) -> p s h d", d=HD),
            )

        def oproj_tile(qc, dt):
            # half-size (one dt chunk) so a single injection stays ~0.9us
            op = scp.tile([128, 2, TB], F32, tag="sc", name="op")
            for cc in range(4):
                nc.tensor.matmul(
                    op[:, 0, :],
                    wo_sb[:, cc, ts(dt, 128)],
                    ao[:, cc, ts(qc, TB)],
                    start=(cc == 0),
                    stop=(cc == 3),
                )
            ost = ostp.tile([128, TB], BF16, tag="ost", name="ost")
            nc.vector.tensor_copy(ost, op[:, 0, :])
            nc.sync.dma_start(out=outT[ts(dt, 128), ts(qc, TB)], in_=ost)

        opart = res.tile([128, 8, TB], F32, tag="opart")

        def oproj3_partial(dt):
            # qc=3 out-proj, head chunks 0..1 only -> SBUF partial
            op = scp.tile([128, 2, TB], F32, tag="sc", name="opp")
            for cc in range(2):
                nc.tensor.matmul(
                    op[:, 0, :],
                    wo_sb[:, cc, ts(dt, 128)],
                    ao[:, cc, ts(3, TB)],
                    start=(cc == 0),
                    stop=(cc == 1),
                )
            nc.vector.tensor_copy(opart[:, dt, :], op[:, 0, :])

        def oproj3_finish(dt):
            # head chunks 2..3 + the saved partial
            op = scp.tile([128, 2, TB], F32, tag="sc", name="opf")
            for cc in range(2, 4):
                nc.tensor.matmul(
                    op[:, 0, :],
                    wo_sb[:, cc, ts(dt, 128)],
                    ao[:, cc, ts(3, TB)],
                    start=(cc == 2),
                    stop=(cc == 3),
                )
            ost = ostp.tile([128, TB], BF16, tag="ost", name="ost")
            nc.vector.tensor_add(ost, op[:, 0, :], opart[:, dt, :])
            nc.sync.dma_start(out=outT[ts(dt, 128), ts(3, TB)], in_=ost)

        # prologue: token block 0 projections for the first head-pair chunk;
        # the dtp=1 chunks are deferred into the j=0 injection stream so
        # attention (and the exp stream) starts as early as possible
        # wo/x(1) go on the same queue strictly behind wv so they cannot
        # jump ahead of it in the DMA lane (wo isn't needed until ~40us in)
        nc.scalar.dma_start(out=wo_sb.rearrange("p c d -> p (c d)"), in_=wod)
        xts[1] = load_x(1, nc.scalar)
        proj_qk_tile(wq_sb, qt8, xts[0], 0, 0)
        proj_qk_tile(wk_sb, kt8, xts[0], 0, 0)
        proj_v_tile(xts[0], 0, 0)

        for j in range(4):
            qc = j
            nktp = 2 * (qc + 1)

            inj = deque()
            if j == 0:
                # deferred prologue tiles: V for block 0 feeds the (lagged)
                # first PVs, the dtp=1 chunks feed hp>=2; hp=0/1 attention
                # (and the exp stream) starts right after Q0/K0
                inj.append(partial(proj_v_tile, xts[0], 0, 1))
                inj.append(partial(proj_qk_tile, wq_sb, qt8, xts[0], 0, 1))
                inj.append(partial(proj_qk_tile, wk_sb, kt8, xts[0], 0, 1))
            if j < 3:
                for dtp in range(2):
                    inj.append(partial(proj_qk_tile, wq_sb, qt8, xts[j + 1], j + 1, dtp))
                for dtp in range(2):
                    inj.append(partial(proj_qk_tile, wk_sb, kt8, xts[j + 1], j + 1, dtp))
                for ttp in range(2):
                    inj.append(partial(proj_v_tile, xts[j + 1], j + 1, ttp))
            if j > 0:
                for dt in range(8):
                    inj.append(partial(oproj_tile, j - 1, dt))
            if j == 3:
                # first-half partial sums of the last block's out-projection
                # (they only need the hp=0/1 chunks of ao, ready mid-block)
                for dt in range(8):
                    inj.append(partial(oproj3_partial, dt))
            if j < 2:
                xts[j + 2] = load_x(j + 2)

            # last iteration: hold one tile back to bridge the final
            # normalize chain before the epilogue out-projection (keeps PE
            # warm through the tail)
            n_inj = len(inj) - 9 if j == 3 else len(inj)
            total_slots = 4 * nktp
            slot = 0
            injected = 0

            for hp in range(4):
                pv = pvp.tile([HDP, 2, TB], F32, tag="pv", name="pv")

                def emit_pv(ets, q0, ktp):
                    for i in range(2):
                        for s in range(2):
                            sq0 = q0 + 128 * s if 2 * ktp >= 4 * qc else q0
                            nc.tensor.matmul(
                                pv[0 : HD + 1, i, sq0:],
                                v8[:, ktp, s, 2 * hp + i, 0 : HD + 1],
                                ets[i][:, s, sq0:],
                                start=(ktp == 0 and s == 0),
                                stop=(ktp == nktp - 1 and s == 1),
                            )

                pending = None  # PVs lag one step so they never wait on exp
                for ktp in range(nktp):
                    rel = 2 * ktp - 4 * qc
                    diag = rel >= 0
                    q0 = rel * 128 if diag else 0
                    qn = TB - q0
                    ets = []
                    for i in range(2):
                        po = 64 * i
                        sc = scp.tile([128, 2, TB], F32, tag="sc", name="sc")
                        for s in range(2):
                            nc.tensor.matmul(
                                sc[:, s, q0:],
                                kt8[po : po + 64, hp, ds(256 * ktp + 128 * s, 128)],
                                qt8[po : po + 64, hp, ds(qc * TB + q0, qn)],
                                start=True,
                                stop=True,
                            )
                        et = etp.tile([128, 2, TB], BF16, tag="et", name="et")
                        nc.scalar.activation(
                            et[:, :, q0:], sc[:, :, q0:], EXP,
                            scale=SC_EXP, bias=nbias,
                        )
                        if diag:
                            nc.vector.tensor_mul(
                                et[:, 0, ds(q0, 128)], et[:, 0, ds(q0, 128)], m1
                            )
                            nc.vector.tensor_mul(
                                et[:, 1, ds(q0, 256)], et[:, 1, ds(q0, 256)], m2
                            )
                        ets.append(et)
                    # paced injection of independent PE work, while ScalarE
                    # runs the exps (and DVE the diagonal masks)
                    slot += 1
                    target = (n_inj * slot + total_slots - 1) // total_slots
                    while inj and injected < target:
                        inj.popleft()()
                        injected += 1
                    if pending is not None:
                        emit_pv(*pending)
                    pending = (ets, q0, ktp)
                emit_pv(*pending)
                # normalize: rows 0..63 / (row 64), per head
                rd = rdp.tile([1, 2, TB], F32, tag="rd", name="rd")
                nc.vector.reciprocal(rd, pv[HD : HD + 1, :, :])
                bcs = bcp.tile([64, 2, TB], F32, tag="bcs", name="bcs")
                nc.gpsimd.partition_broadcast(bcs, rd)
                for i in range(2):
                    po = 64 * i
                    nc.vector.tensor_mul(
                        ao[po : po + 64, hp, ts(qc, TB)], pv[0:HD, i, :], bcs[:, i, :]
                    )
            while inj:
                inj.popleft()()

        # epilogue: finish the last q block's output projection
        for dt in range(8):
            oproj3_finish(dt)

    nc.compile()
    return nc


def _make_masks():
    # M1: lower-triangular 128x128 (valid iff q >= k); M2: [zeros | M1]
    k = np.arange(128)[:, None]
    q = np.arange(128)[None, :]
    m1 = (q >= k).astype(np.float32)
    m = np.zeros((128, 384), np.float32)
    m[:, 0:128] = m1
    m[:, 256:384] = m1
    return m


def kernel(x, Wq, bq, Wk, bk, Wv, bv, Wo, bo):
    global LAST_RESULTS
    import ml_dtypes

    F8NP = ml_dtypes.float8_e4m3
    BF16NP = ml_dtypes.bfloat16

    x = np.asarray(x, np.float32)
    Wq, bq = np.asarray(Wq, np.float32), np.asarray(bq, np.float32)
    Wk, bk = np.asarray(Wk, np.float32), np.asarray(bk, np.float32)
    Wv, bv = np.asarray(Wv, np.float32), np.asarray(bv, np.float32)
    Wo, bo = np.asarray(Wo, np.float32), np.asarray(bo, np.float32)

    with_bias = bool(np.any(bq) or np.any(bk) or np.any(bv))
    npair = 5 if with_bias else 4
    deff = 256 * npair

    masks = _make_masks().astype(BF16NP)

    def split8(a):
        # fp8 value + fp8 residual at the same scale
        hi = a.astype(F8NP)
        lo = (a - hi.astype(np.float32)).astype(F8NP)
        return hi, lo

    def aug_x(xb):
        # xT (+ ones row at 1024 when biased), pre-tiled to [4, 2, 128, nd*TB]
        xa = np.zeros((deff, T), np.float32)
        xa[:D] = xb.T
        if with_bias:
            xa[D] = 1.0
        # row c*128+p, col tb*TB+t -> [tb, p, c*TB+t]
        xt = xa.reshape(2 * npair, 128, 4, TB).transpose(2, 1, 0, 3)
        xt = np.ascontiguousarray(xt.reshape(4, 128, 2 * npair * TB))
        hi, lo = split8(xt)
        return np.ascontiguousarray(np.stack([hi, lo], axis=1))

    def aug_w(W, b, sl):
        wa = np.zeros((deff, DH), np.float32)
        wa[:D] = W[:, sl] * WSCALE
        if with_bias:
            wa[D] = b[sl] * WSCALE
        wt = wa.reshape(2 * npair, 128, DH).transpose(1, 0, 2)
        wt = np.ascontiguousarray(wt.reshape(128, 2 * npair * DH))
        hi, lo = split8(wt)
        return np.ascontiguousarray(np.stack([hi, lo], axis=0))

    in_maps = []
    for c in range(NCORES):
        b, hh = c // 2, c % 2
        sl = slice(hh * DH, (hh + 1) * DH)
        in_maps.append(
            {
                "x8": aug_x(x[b]),
                "wq8": aug_w(Wq, bq, sl),
                "wk8": aug_w(Wk, bk, sl),
                "wv8": aug_w(Wv, bv, sl),
                "wo16": np.ascontiguousarray(
                    Wo[sl, :].reshape(4, 128, D).transpose(1, 0, 2).reshape(128, 4 * D)
                ).astype(BF16NP),
                "masks": masks,
            }
        )

    nc = _build_nc(npair)
    res = run_bass_kernel_spmd(
        nc,
        in_maps,
        core_ids=list(range(NCORES)),
        trace=bool(int(os.environ.get("KERNEL_TRACE", "0"))),
    )
    LAST_RESULTS = res

    out = np.empty((B, T, D), np.float32)
    for b in range(B):
        acc = (
            res.results[2 * b]["outT"].astype(np.float32)
            + res.results[2 * b + 1]["outT"].astype(np.float32)
        )
        out[b] = acc.T + bo
    return out


# revision 56
# speedup vs baseline: 1.0228x; 1.0113x over previous
"""GQA (n_group == n_head) causal attention kernel for 8 Trainium2 NeuronCores.

Sharding: core c -> (batch b = c//2, head-half hh = c%2).  Each core computes
Q/K/V projections for its 8 heads over the full sequence, causal attention,
and a partial output projection against its 512 rows of Wo.  The host sums
the two partial outputs per batch (the tensor-parallel reduce), adds bo, and
transposes back.

Precision / engine plan:
  - Q/K/V projections: fp8(e4m3) DoubleRow matmuls in THREE passes
    (x_a@W_a + x_a@W_r + x_r@W_a, where _a = fp8 rounding of the 16x-scaled
    operand and _r = fp8 of its rounding residual at the same scale).  This
    recovers ~bf16-level projection accuracy at half the bf16 matmul cost.
    The 16x weight prescale cancels in softmax (Q*K picks up 16^2 -> folded
    into the exp scale; V's 16 cancels against a denominator ones-column of
    value 16).
  - QK^T: bf16, scoresT tiles [k=128, 2 kt-slots, q<=512] in PSUM.
  - exp: ScalarE, PSUM f32 -> SBUF bf16, scale=0.125/256.
  - causal mask: bf16 multiplies on DVE over the diagonal corners only.
  - PV: bf16 against V~ = [16*V | 16] (row 64 = 16*denominator).
  - normalize: DVE reciprocal + Pool partition_broadcast + DVE multiply.
  - out-proj: bf16 matmuls, DVE copy to SBUF, DMA to DRAM.  bo on the host.

Schedule: the ScalarE exp stream is the critical resource (~154us of work),
so projection tiles for token block j+1 and the out-projection for q block
j-1 are injected one PSUM-tile at a time between attention steps of q block
j, keeping both PE and ScalarE continuously busy.

Nonzero bq/bk/bv are supported by augmenting x with a ones row and the
weights with a bias row (npair=5); the staged problem has zero biases so the
default build uses npair=4.
"""

import os
from collections import deque
from contextlib import ExitStack
from functools import partial

import numpy as np

import concourse.bass as bass
import concourse.mybir as mybir
import concourse.tile as tile
from concourse import bacc
from concourse.bass import ds, ts
from concourse.bass_utils import run_bass_kernel_spmd

B, T, D = 4, 2048, 1024
H, HD = 16, 64
NCORES = 8
HH = H // 2            # heads per core = 8
DH = HH * HD           # head dims per core = 512
TB = 512               # token block (q block, proj block)
WSCALE = 16.0          # host-side weight prescale (fp8 range)
SC_EXP = 0.125 / (WSCALE * WSCALE)   # exp input scale for scores
EXP_BIAS = 0.0         # bf16 exp output needs no range shaping

F32 = mybir.dt.float32
BF16 = mybir.dt.bfloat16
F8 = mybir.dt.float8e4
DR = mybir.MatmulPerfMode.DoubleRow
EXP = mybir.ActivationFunctionType.Exp
HDP = HD + 2           # V~ row dim padded so bf16 head strides stay 4-byte
                       # aligned for ldweights (row 64 = denominator ones)

LAST_RESULTS = None


def _build_nc(npair=4):
    deff = 256 * npair
    nc = bacc.Bacc(
        "TRN2",
        target_bir_lowering=False,
        debug=False,
        enable_asserts=False,
        num_devices=NCORES,
    )

    # all big inputs are pre-arranged on the host to match their SBUF
    # layout exactly: one contiguous DMA per load, no gather patterns
    # index 0 = fp8 rounding of the operand, 1 = fp8 of its residual
    x8d = nc.dram_tensor("x8", [4, 2, 128, 2 * npair * TB], F8, kind="ExternalInput").ap()
    wqd = nc.dram_tensor("wq8", [2, 128, 2 * npair * DH], F8, kind="ExternalInput").ap()
    wkd = nc.dram_tensor("wk8", [2, 128, 2 * npair * DH], F8, kind="ExternalInput").ap()
    wvd = nc.dram_tensor("wv8", [2, 128, 2 * npair * DH], F8, kind="ExternalInput").ap()
    wod = nc.dram_tensor("wo16", [128, 4 * D], BF16, kind="ExternalInput").ap()
    maskd = nc.dram_tensor("masks", [128, 384], BF16, kind="ExternalInput").ap()
    outT = nc.dram_tensor("outT", [D, T], BF16, kind="ExternalOutput").ap()

    nd = 2 * npair

    with tile.TileContext(nc) as tc, ExitStack() as ctx:
        res = ctx.enter_context(tc.tile_pool(name="res", bufs=1))
        qt8 = res.tile([128, 4, T], BF16, tag="qt")     # 16*Q^T: chunk hp, head i at partitions 64i+
        kt8 = res.tile([128, 4, T], BF16, tag="kt")
        v8 = res.tile([128, 8, 2, HH, HDP], BF16, tag="v")  # [k, ktp, slot, h, 16*V|16|pad]
        ao = res.tile([128, 4, T], BF16, tag="ao")      # attn outT (normalized)
        wo_sb = res.tile([128, 4, D], BF16, tag="wo")
        wq_sb = res.tile([128, 2, nd, DH], F8, tag="wq")
        wk_sb = res.tile([128, 2, nd, DH], F8, tag="wk")
        wv_sb = res.tile([128, 2, nd, DH], F8, tag="wv")
        mask_sb = res.tile([128, 384], BF16, tag="mask")
        nbias = res.tile([128, 1], F32, tag="nbias")
        warm = res.tile([1, 1], F32, tag="warm")

        m1 = mask_sb[:, 0:128]
        m2 = mask_sb[:, 128:384]

        scp = ctx.enter_context(tc.tile_pool(name="scp", bufs=2, space="PSUM"))
        pvp = ctx.enter_context(tc.tile_pool(name="pvp", bufs=2, space="PSUM"))
        xp = ctx.enter_context(tc.tile_pool(name="xp", bufs=2))
        etp = ctx.enter_context(tc.tile_pool(name="etp", bufs=6))
        rdp = ctx.enter_context(tc.tile_pool(name="rdp", bufs=2))
        bcp = ctx.enter_context(tc.tile_pool(name="bcp", bufs=2))
        ostp = ctx.enter_context(tc.tile_pool(name="ostp", bufs=6))

        def load_x(tb, engine=None):
            xt = xp.tile([128, 2, nd, TB], F8, tag="xt", name=f"xt{tb}")
            eng = engine or nc.gpsimd
            for half in range(2):
                eng.dma_start(
                    out=xt[:, half].rearrange("p c t -> p (c t)"),
                    in_=x8d[tb, half],
                )
            return xt

        nc.gpsimd.memset(nbias, EXP_BIAS)
        # pay the Exp activation-table load during the DMA prologue
        nc.scalar.activation(warm, nbias[0:1, :], EXP, scale=1.0)
        # prologue loads: x(0)/wq on the fast HWDGE queues first (they gate
        # the first projection), everything else behind them
        for half in range(2):
            nc.sync.dma_start(
                out=wq_sb[:, half].rearrange("p c d -> p (c d)"), in_=wqd[half]
            )
        xts = {0: load_x(0, nc.scalar)}
        for half in range(2):
            nc.scalar.dma_start(
                out=wk_sb[:, half].rearrange("p c d -> p (c d)"), in_=wkd[half]
            )
        for half in range(2):
            nc.scalar.dma_start(
                out=wv_sb[:, half].rearrange("p c d -> p (c d)"), in_=wvd[half]
            )
        nc.sync.dma_start(out=mask_sb, in_=maskd)
        # ones column of V~ (value 16, matches the 16*V scaling); the pad
        # rows 65..67 just need to be written with something finite
        nc.gpsimd.memset(v8[:, :, :, :, HD:HDP], WSCALE)

        # PE warm-up: keep the PE busy on throwaway matmuls while the first
        # DMAs land, so real work starts at full clock (p-state ramp)
        junk = res.tile([128, TB], BF16, tag="junk")
        junkw = res.tile([128, 1], BF16, tag="junkw")
        nc.gpsimd.memset(junk, 0.0)
        nc.gpsimd.memset(junkw, 0.0)
        for _ in range(6):
            wps = scp.tile([128, 2, TB], F32, tag="sc", name="wps")
            nc.tensor.matmul(wps[0:1, 0, :], junkw, junk, start=True, stop=True)

        PASSES = ((0, 0), (1, 0), (0, 1))  # (w half, x half): a*a + r*a + a*r

        def proj_qk_tile(w_sb, dst, xt, j, dtp):
            ps = scp.tile([128, 2, TB], F32, tag="sc", name="psqk")
            for half in range(2):
                dt = 2 * dtp + half
                for pi, (wh, xh) in enumerate(PASSES):
                    for p in range(npair):
                        nc.tensor.matmul(
                            ps[:, half, :],
                            w_sb[:, wh, 2 * p : 2 * p + 2, ts(dt, 128)],
                            xt[:, xh, 2 * p : 2 * p + 2, :],
                            start=(pi == 0 and p == 0),
                            stop=(pi == 2 and p == npair - 1),
                            perf_mode=DR,
                        )
            nc.vector.tensor_copy(dst[:, 2 * dtp : 2 * dtp + 2, ts(j, TB)], ps)

        def proj_v_tile(xt, j, ttp):
            ps = scp.tile([128, 2, TB], F32, tag="sc", name="psv")
            for half in range(2):
                tt = 2 * ttp + half
                for pi, (wh, xh) in enumerate(PASSES):
                    for p in range(npair):
                        nc.tensor.matmul(
                            ps[:, half, :],
                            xt[:, xh, 2 * p : 2 * p + 2, ts(tt, 128)],
                            wv_sb[:, wh, 2 * p : 2 * p + 2, :],
                            start=(pi == 0 and p == 0),
                            stop=(pi == 2 and p == npair - 1),
                            perf_mode=DR,
                        )
            nc.vector.tensor_copy(
                v8[:, 2 * j + ttp, :, :, 0:HD],
                ps.rearrange("p s (h d) -> p s h d", d=HD),
            )

        def oproj_tile(qc, dt):
            # half-size (one dt chunk) so a single injection stays ~0.9us
            op = scp.tile([128, 2, TB], F32, tag="sc", name="op")
            for cc in range(4):
                nc.tensor.matmul(
                    op[:, 0, :],
                    wo_sb[:, cc, ts(dt, 128)],
                    ao[:, cc, ts(qc, TB)],
                    start=(cc == 0),
                    stop=(cc == 3),
                )
            ost = ostp.tile([128, TB], BF16, tag="ost", name="ost")
            nc.vector.tensor_copy(ost, op[:, 0, :])
            nc.sync.dma_start(out=outT[ts(dt, 128), ts(qc, TB)], in_=ost)

        opart = res.tile([128, 8, TB], F32, tag="opart")

        def oproj3_partial(dt):
            # qc=3 out-proj, head chunks 0..1 only -> SBUF partial
            op = scp.tile([128, 2, TB], F32, tag="sc", name="opp")
            for cc in range(2):
                nc.tensor.matmul(
                    op[:, 0, :],
                    wo_sb[:, cc, ts(dt, 128)],
                    ao[:, cc, ts(3, TB)],
                    start=(cc == 0),
                    stop=(cc == 1),
                )
            nc.vector.tensor_copy(opart[:, dt, :], op[:, 0, :])

        def oproj3_finish(dt):
            # head chunks 2..3 + the saved partial
            op = scp.tile([128, 2, TB], F32, tag="sc", name="opf")
            for cc in range(2, 4):
                nc.tensor.matmul(
                    op[:, 0, :],
                    wo_sb[:, cc, ts(dt, 128)],
                    ao[:, cc, ts(3, TB)],
                    start=(cc == 2),
                    stop=(cc == 3),
                )
            ost = ostp.tile([128, TB], BF16, tag="ost", name="ost")
            nc.vector.tensor_add(ost, op[:, 0, :], opart[:, dt, :])
            nc.sync.dma_start(out=outT[ts(dt, 128), ts(3, TB)], in_=ost)

        # prologue: token block 0 projections for the first head-pair chunk;
        # the dtp=1 chunks are deferred into the j=0 injection stream so
        # attention (and the exp stream) starts as early as possible
        # wo/x(1) go on the same queue strictly behind wv so they cannot
        # jump ahead of it in the DMA lane (wo isn't needed until ~40us in)
        nc.scalar.dma_start(out=wo_sb.rearrange("p c d -> p (c d)"), in_=wod)
        xts[1] = load_x(1, nc.scalar)
        proj_qk_tile(wq_sb, qt8, xts[0], 0, 0)
        proj_qk_tile(wk_sb, kt8, xts[0], 0, 0)
        proj_v_tile(xts[0], 0, 0)

        for j in range(4):
            qc = j
            nktp = 2 * (qc + 1)

            inj = deque()
            if j == 0:
                # deferred prologue tiles: V for block 0 feeds the (lagged)
                # first PVs, the dtp=1 chunks feed hp>=2; hp=0/1 attention
                # (and the exp stream) starts right after Q0/K0
                inj.append(partial(proj_v_tile, xts[0], 0, 1))
                inj.append(partial(proj_qk_tile, wq_sb, qt8, xts[0], 0, 1))
                inj.append(partial(proj_qk_tile, wk_sb, kt8, xts[0], 0, 1))
            if j < 3:
                for dtp in range(2):
                    inj.append(partial(proj_qk_tile, wq_sb, qt8, xts[j + 1], j + 1, dtp))
                for dtp in range(2):
                    inj.append(partial(proj_qk_tile, wk_sb, kt8, xts[j + 1], j + 1, dtp))
                for ttp in range(2):
                    inj.append(partial(proj_v_tile, xts[j + 1], j + 1, ttp))
            if j > 0:
                for dt in range(8):
                    inj.append(partial(oproj_tile, j - 1, dt))
            if j == 3:
                # first-half partial sums of the last block's out-projection
                # (they only need the hp=0/1 chunks of ao, ready mid-block)
                for dt in range(8):
                    inj.append(partial(oproj3_partial, dt))
            if j < 2:
                xts[j + 2] = load_x(j + 2)

            # last iteration: hold one tile back to bridge the final
            # normalize chain before the epilogue out-projection (keeps PE
            # warm through the tail)
            n_inj = len(inj) - {0: 0, 1: 0, 2: 0, 3: 16}[j]
            total_slots = 4 * nktp
            slot = 0
            injected = 0

            for hp in range(4):
                pv = pvp.tile([HDP, 2, TB], F32, tag="pv", name="pv")

                def emit_pv(ets, q0, ktp):
                    for i in range(2):
                        for s in range(2):
                            sq0 = q0 + 128 * s if 2 * ktp >= 4 * qc else q0
                            nc.tensor.matmul(
                                pv[0 : HD + 1, i, sq0:],
                                v8[:, ktp, s, 2 * hp + i, 0 : HD + 1],
                                ets[i][:, s, sq0:],
                                start=(ktp == 0 and s == 0),
                                stop=(ktp == nktp - 1 and s == 1),
                            )

                pending = None  # PVs lag one step so they never wait on exp
                for ktp in range(nktp):
                    rel = 2 * ktp - 4 * qc
                    diag = rel >= 0
                    q0 = rel * 128 if diag else 0
                    qn = TB - q0
                    ets = []
                    for i in range(2):
                        po = 64 * i
                        sc = scp.tile([128, 2, TB], F32, tag="sc", name="sc")
                        for s in range(2):
                            nc.tensor.matmul(
                                sc[:, s, q0:],
                                kt8[po : po + 64, hp, ds(256 * ktp + 128 * s, 128)],
                                qt8[po : po + 64, hp, ds(qc * TB + q0, qn)],
                                start=True,
                                stop=True,
                            )
                        et = etp.tile([128, 2, TB], BF16, tag="et", name="et")
                        nc.scalar.activation(
                            et[:, :, q0:], sc[:, :, q0:], EXP,
                            scale=SC_EXP, bias=nbias,
                        )
                        if diag:
                            nc.vector.tensor_mul(
                                et[:, 0, ds(q0, 128)], et[:, 0, ds(q0, 128)], m1
                            )
                            nc.vector.tensor_mul(
                                et[:, 1, ds(q0, 256)], et[:, 1, ds(q0, 256)], m2
                            )
                        ets.append(et)
                    # paced injection of independent PE work, while ScalarE
                    # runs the exps (and DVE the diagonal masks)
                    slot += 1
                    target = (n_inj * slot + total_slots - 1) // total_slots
                    while inj and injected < target:
                        inj.popleft()()
                        injected += 1
                    if pending is not None:
                        emit_pv(*pending)
                    pending = (ets, q0, ktp)
                emit_pv(*pending)
                # normalize: rows 0..63 / (row 64), per head
                rd = rdp.tile([1, 2, TB], F32, tag="rd", name="rd")
                nc.vector.reciprocal(rd, pv[HD : HD + 1, :, :])
                bcs = bcp.tile([64, 2, TB], F32, tag="bcs", name="bcs")
                nc.gpsimd.partition_broadcast(bcs, rd)
                for i in range(2):
                    po = 64 * i
                    nc.vector.tensor_mul(
                        ao[po : po + 64, hp, ts(qc, TB)], pv[0:HD, i, :], bcs[:, i, :]
                    )
            while inj:
                inj.popleft()()

        # epilogue: finish the last q block's output projection
        for dt in range(8):
            oproj3_finish(dt)

    nc.compile()
    return nc


def _make_masks():
    # M1: lower-triangular 128x128 (valid iff q >= k); M2: [zeros | M1]
    k = np.arange(128)[:, None]
    q = np.arange(128)[None, :]
    m1 = (q >= k).astype(np.float32)
    m = np.zeros((128, 384), np.float32)
    m[:, 0:128] = m1
    m[:, 256:384] = m1
    return m


def kernel(x, Wq, bq, Wk, bk, Wv, bv, Wo, bo):
    global LAST_RESULTS
    import ml_dtypes

    F8NP = ml_dtypes.float8_e4m3
    BF16NP = ml_dtypes.bfloat16

    x = np.asarray(x, np.float32)
    Wq, bq = np.asarray(Wq, np.float32), np.asarray(bq, np.float32)
    Wk, bk = np.asarray(Wk, np.float32), np.asarray(bk, np.float32)
    Wv, bv = np.asarray(Wv, np.float32), np.asarray(bv, np.float32)
    Wo, bo = np.asarray(Wo, np.float32), np.asarray(bo, np.float32)

    with_bias = bool(np.any(bq) or np.any(bk) or np.any(bv))
    npair = 5 if with_bias else 4
    deff = 256 * npair

    masks = _make_masks().astype(BF16NP)

    def split8(a):
        # fp8 value + fp8 residual at the same scale
        hi = a.astype(F8NP)
        lo = (a - hi.astype(np.float32)).astype(F8NP)
        return hi, lo

    def aug_x(xb):
        # xT (+ ones row at 1024 when biased), pre-tiled to [4, 2, 128, nd*TB]
        xa = np.zeros((deff, T), np.float32)
        xa[:D] = xb.T
        if with_bias:
            xa[D] = 1.0
        # row c*128+p, col tb*TB+t -> [tb, p, c*TB+t]
        xt = xa.reshape(2 * npair, 128, 4, TB).transpose(2, 1, 0, 3)
        xt = np.ascontiguousarray(xt.reshape(4, 128, 2 * npair * TB))
        hi, lo = split8(xt)
        return np.ascontiguousarray(np.stack([hi, lo], axis=1))

    def aug_w(W, b, sl):
        wa = np.zeros((deff, DH), np.float32)
        wa[:D] = W[:, sl] * WSCALE
        if with_bias:
            wa[D] = b[sl] * WSCALE
        wt = wa.reshape(2 * npair, 128, DH).transpose(1, 0, 2)
        wt = np.ascontiguousarray(wt.reshape(128, 2 * npair * DH))
        hi, lo = split8(wt)
        return np.ascontiguousarray(np.stack([hi, lo], axis=0))

    in_maps = []
    for c in range(NCORES):
        b, hh = c // 2, c % 2
        sl = slice(hh * DH, (hh + 1) * DH)
        in_maps.append(
            {
                "x8": aug_x(x[b]),
                "wq8": aug_w(Wq, bq, sl),
                "wk8": aug_w(Wk, bk, sl),
                "wv8": aug_w(Wv, bv, sl),
                "wo16": np.ascontiguousarray(
                    Wo[sl, :].reshape(4, 128, D).transpose(1, 0, 2).reshape(128, 4 * D)
                ).astype(BF16NP),
                "masks": masks,
            }
        )

    nc = _build_nc(npair)
    res = run_bass_kernel_spmd(
        nc,
        in_maps,
        core_ids=list(range(NCORES)),
        trace=bool(int(os.environ.get("KERNEL_TRACE", "0"))),
    )
    LAST_RESULTS = res

    out = np.empty((B, T, D), np.float32)
    for b in range(B):
        acc = (
            res.results[2 * b]["outT"].astype(np.float32)
            + res.results[2 * b + 1]["outT"].astype(np.float32)
        )
        out[b] = acc.T + bo
    return out
